# revision 11
# baseline (speedup 1.0000x reference)
"""Bass/Tile TRN2 kernel for nn_Encoder_55233279426649 (dual-stream encoder
block with cross-stream attention-map conv).

Sharding: data-parallel over batch — 32 batches -> 8 NeuronCores x 4 batches.
Inside each core everything runs in "feature-major" (transposed) layouts so
only host-side transposes are needed. All matmuls run in bf16 on the PE
(fp32 PSUM accumulation); the attention-map conv block is decomposed as
  sup = mu*(A*D0 + B*D1 + Cd) + nu*sum_o w2_o |z_o| + c2,   mu=(1+l)/2, nu=(1-l)/2
(LeakyReLU(z) = mu*z + nu*|z|), evaluated with fused DVE ops over
batch-packed tiles. Softmax has no max-subtract (logits are O(1)); the
denominator comes free from an appended ones-column in V.
"""
import hashlib
import numpy as np
import ml_dtypes

import concourse.bass as bass
import concourse.mybir as mybir
import concourse.tile as tile
from concourse.vector_clock import ScopedClock
from concourse.bass_utils import run_bass_kernel_spmd

# ---------------------------------------------------------------- constants
B, N, D, H, NA = 32, 257, 512, 8, 8
DH = D // H
SCALE = (D / H) ** -0.5
BN_EPS = 1e-5
LAM = 0.01
MU = (1 + LAM) / 2
NU = (1 - LAM) / 2
NP_ = 258            # padded query length (even for DVE packed modes)
NCORES = 8
BPC = B // NCORES    # batches per core
F32 = mybir.dt.float32
BF16 = mybir.dt.bfloat16
AF = mybir.ActivationFunctionType
OP = mybir.AluOpType

# ------------------------------------------------- walrus 1-wait legalizer
_ctr = [0]


def _mk_wait_nop(engine, wait):
    _ctr[0] += 1
    nop = mybir.InstNoOp(name=f"Iws-{_ctr[0]}", engine=engine, ins=[], outs=[])
    nop.sync_info = mybir.SyncInfo(on_wait=[wait], on_update=[])
    return nop


class FixedTileContext(tile.TileContext):
    """Splits >1-wait instructions into wait-carrying nops (this container's
    walrus accepts at most one sync-wait command per instruction)."""

    def _lower_ordered_insts(self, postordered_blocks):
        for bb_name in list(postordered_blocks.keys()):
            insts = postordered_blocks[bb_name]
            new = []
            changed = False
            for inst in insts:
                si = inst.sync_info
                if si is not None and si.on_wait is not None and len(si.on_wait) > 1:
                    waits = list(si.on_wait)
                    for w in waits[:-1]:
                        new.append(_mk_wait_nop(inst.engine, w))
                    si.on_wait = [waits[-1]]
                    changed = True
                new.append(inst)
            if changed:
                if isinstance(insts, list):
                    insts[:] = new
                else:
                    postordered_blocks[bb_name] = new
        return super()._lower_ordered_insts(postordered_blocks)

    def _drain_and_barrier(self, tick_clock, wait_clock):
        nc = self.nc
        drain_inst = nc.sync.drain()
        wait_clock.add_sem_waits(
            drain_inst.ins, ScopedClock({None: tick_clock.global_clock})
        )
        si = drain_inst.ins.sync_info
        if si is not None and si.on_wait is not None and len(si.on_wait) > 1:
            waits = list(si.on_wait)
            si.on_wait = waits[:1]
            for w in waits[1:]:
                d2 = nc.sync.drain()
                si2 = d2.ins.sync_info
                if si2 is None:
                    d2.ins.sync_info = mybir.SyncInfo(on_wait=[w], on_update=[])
                else:
                    si2.on_wait = list(si2.on_wait or []) + [w]
        nc.all_engine_barrier()
        assert self.sems is not None
        popped = nc._tile_sem_poison_stack.pop()
        assert popped is self._sem_poison
        nc.clear_and_free_semaphores(list(self.sems.allocated().values()))
        nc.all_engine_barrier()


# ------------------------------------------------------------- host folding
def _fold_consts(inputs):
    """Returns dict of host-folded constants (f64 where it matters)."""
    conv1_w = np.asarray(inputs['conv1_w'], np.float64)
    conv1_b = np.asarray(inputs['conv1_b'], np.float64)
    bn_g = np.asarray(inputs['bn_g'], np.float64)
    bn_b = np.asarray(inputs['bn_b'], np.float64)
    conv2_w = np.asarray(inputs['conv2_w'], np.float64)
    conv2_b = np.asarray(inputs['conv2_b'], np.float64)
    inv = 1.0 / np.sqrt(1.0 + BN_EPS)
    g = conv1_w[:, :, 0] * bn_g * inv          # [H, NA] coef on D0 (dots)
    h = conv1_w[:, :, 1] * bn_g * inv          # coef on D1 (dots1)
    d = conv1_b * bn_g * inv + bn_b            # [H, NA]
    w2 = conv2_w
    A = (w2 * g).sum(1)
    Bc = (w2 * h).sum(1)
    Cd = (w2 * d).sum(1)
    const0 = MU * Cd + conv2_b                 # exp bias per head
    eps = 1e-30
    piv_is_h = np.abs(h) >= np.abs(g)
    hs = np.where(np.abs(h) < eps, eps, h)
    gs = np.where(np.abs(g) < eps, eps, g)
    rho = np.where(piv_is_h, g / hs, h / gs)
    m = np.where(piv_is_h, hs, gs)
    chat = NU * w2
    As = np.where(np.abs(A) < eps, eps, A)
    Bs = np.where(np.abs(Bc) < eps, eps, Bc)
    piv9_is_B = np.abs(Bc) >= np.abs(A)
    rho9 = np.where(piv9_is_B, A / Bs, Bc / As)
    m9 = np.where(piv9_is_B, Bs, As) * MU
    # ragged (fixed pivot = h / B)
    rho_r = g / hs
    m_r = hs
    rho9_r = A / Bs
    m9_r = Bs * MU
    # full units use the relu form: LReLU(z) = lam*z + (1-lam)*relu(z)
    c_full = (1 - LAM) * w2                       # [H, NA]
    M_full = c_full * m                           # scalar1 for opB
    s2_full = -c_full * d                         # scalar2 for opB (max/min shift)
    is_max = w2 >= 0
    m9_lam = np.where(piv9_is_B, Bs, As) * LAM
    const0_full = LAM * Cd + conv2_b + (c_full * d).sum(1)
    return dict(g=g, h=h, d=d, piv_is_h=piv_is_h, rho=rho, m=m, chat=chat,
                rho9=rho9, m9=m9, piv9_is_B=piv9_is_B, const0=const0,
                rho_r=rho_r, m_r=m_r, rho9_r=rho9_r, m9_r=m9_r,
                M_full=M_full, s2_full=s2_full, is_max=is_max,
                m9_lam=m9_lam, const0_full=const0_full)


# ------------------------------------------------------------- bass builder
def _build(cc):
    """cc: dict of folded conv consts (floats embedded as immediates)."""
    nc = bass.Bass()
    xt = nc.dram_tensor("xt", [2, BPC, 4, 128, NP_], BF16, kind="ExternalInput")
    wqk = nc.dram_tensor("wqk", [2, 4, 128, 1024], BF16, kind="ExternalInput")
    wv = nc.dram_tensor("wv", [2, 4, 128, 512], BF16, kind="ExternalInput")
    wm = nc.dram_tensor("wm", [2, 4, 128, 512], BF16, kind="ExternalInput")
    qkb = nc.dram_tensor("qkb", [128, 2, 8], F32, kind="ExternalInput")
    bmv = nc.dram_tensor("bmv", [128, 2, 4], F32, kind="ExternalInput")
    cvc = nc.dram_tensor("cvc", [8, 28], F32, kind="ExternalInput")
    res = nc.dram_tensor("res", [2, BPC, 4, 128, NP_], F32, kind="ExternalOutput")

    with FixedTileContext(nc) as tc:
        konst = tc.alloc_tile_pool(name="konst", bufs=1)
        ppool = tc.alloc_tile_pool(name="ppool", bufs=8, space="PSUM")
        xpool = tc.alloc_tile_pool(name="xpool", bufs=2)
        dpool = tc.alloc_tile_pool(name="dpool", bufs=2)
        tpool = tc.alloc_tile_pool(name="tpool", bufs=2)
        rpool = tc.alloc_tile_pool(name="rpool", bufs=2)

        # ---- resident constants/weights
        qkb_sb = konst.tile([128, 2, 8], F32, name="qkb_sb")
        nc.sync.dma_start(out=qkb_sb, in_=qkb[:, :, :])
        bmv_sb = konst.tile([128, 2, 4], F32, name="bmv_sb")
        nc.sync.dma_start(out=bmv_sb, in_=bmv[:, :, :])
        cvc_sb = konst.tile([8, 28], F32, name="cvc_sb")
        nc.sync.dma_start(out=cvc_sb, in_=cvc[:, :])
        ones_sb = konst.tile([1, 64], BF16, name="ones_sb")
        nc.vector.memset(ones_sb, 1.0)
        onesf_sb = konst.tile([1, 64], F32, name="onesf_sb")
        nc.vector.memset(onesf_sb, 1.0)

        QK_sb = konst.tile([128, BPC, 2, 8, NP_], BF16, name="QK_sb")
        v_sb = konst.tile([128, BPC, 2, 3, 8, 65], BF16, name="v_sb")
        outT_sb = konst.tile([128, BPC, 2, 4, NP_], BF16, name="outT_sb")
        rag_sb = konst.tile([40, 2, BPC, NP_], BF16, name="rag_sb")
        Drag_sb = rag_sb[0:8]
        Erag_sb = rag_sb[32:40]

        # ---- phase 1: projections (s-outer so weights stream once per s)
        for s in range(2):
            wqk_sb = xpool.tile([128, 4, 1024], BF16, name="wqk_sb", tag="wqk", bufs=1)
            nc.sync.dma_start(out=wqk_sb, in_=wqk[s].rearrange("t p f -> p t f"))
            wv_sb = xpool.tile([128, 4, 512], BF16, name="wv_sb", tag="wv", bufs=1)
            nc.sync.dma_start(out=wv_sb, in_=wv[s].rearrange("t p f -> p t f"))
            for b in range(BPC):
                xts = xpool.tile([128, 4, NP_], BF16, name="xts", tag="xts")
                nc.sync.dma_start(out=xts, in_=xt[s, b].rearrange("t p i -> p t i"))
                for ft in range(8):
                    ps = ppool.tile([128, NP_], F32, name="psqk", tag="ps")
                    for dt in range(4):
                        nc.tensor.matmul(
                            ps, lhsT=wqk_sb[:, dt, ft * 128:(ft + 1) * 128],
                            rhs=xts[:, dt, :], start=(dt == 0), stop=(dt == 3))
                    nc.scalar.activation(QK_sb[:, b, s, ft, :], ps, AF.Identity,
                                         bias=qkb_sb[:, s, ft:ft + 1], scale=1.0)
                for tt in range(3):
                    tsz = 128 if tt < 2 else 1
                    ps = ppool.tile([128, 512], F32, name="psv", tag="ps")
                    for dt in range(4):
                        nc.tensor.matmul(
                            ps[0:tsz, :], lhsT=xts[:, dt, tt * 128:tt * 128 + tsz],
                            rhs=wv_sb[:, dt, :], start=(dt == 0), stop=(dt == 3))
                    nc.vector.tensor_copy(v_sb[0:tsz, b, s, tt, :, 0:64],
                                          ps[0:tsz, :].rearrange("p (h e) -> p h e", h=8))
                    nc.vector.memset(v_sb[0:tsz, b, s, tt, :, 64:65], 1.0)

        # ---- phase 2a: ragged (j=256) dots for all heads -> Drag
        Dragf_sb = konst.tile([1, 8, 2, BPC, NP_], BF16, name="Dragf_sb")
        for b in range(BPC):
            for s in range(2):
                for hh in range(8):
                    p0 = (hh % 2) * 64
                    kft = 4 + hh // 2
                    qft = hh // 2
                    psr = ppool.tile([1, NP_], F32, name="psr", tag="ps")
                    nc.tensor.matmul(
                        psr,
                        lhsT=QK_sb[p0:p0 + 64, b, s, kft, 256:257],
                        rhs=QK_sb[p0:p0 + 64, b, s, qft, :],
                        start=True, stop=True)
                    nc.scalar.activation(Dragf_sb[0:1, hh, s, b, :], psr, AF.Copy,
                                         bias=0.0, scale=1.0)
        for hh in range(8):
            nc.sync.dma_start(out=Drag_sb[hh:hh + 1], in_=Dragf_sb[0:1, hh])

        # ---- phase 2b: ragged conv unit (per-partition consts over heads)
        def conv_unit(D0ap, D1ap, outEap, ragged):
            """D0ap/D1ap: [P, BPC, NP_] bf16 views. outEap(s) -> E view."""
            if ragged:
                def sc(col):
                    return cvc_sb[:, col:col + 1]
                rho_o = [sc(o) for o in range(8)]
                m_o = [sc(8 + o) for o in range(8)]
                chat_o = [sc(16 + o) for o in range(8)]
                rho9v, m9v, expb = sc(24), sc(25), sc(26)
                d_o = [sc(27) for o in range(8)]
                piv = [True] * 8
                piv9 = True
            else:
                hh = conv_unit.cur_h
                rho_o = [float(cc['rho'][hh, o]) for o in range(8)]
                M_o = [float(cc['M_full'][hh, o]) for o in range(8)]
                s2_o = [float(cc['s2_full'][hh, o]) for o in range(8)]
                ismax_o = [bool(cc['is_max'][hh, o]) for o in range(8)]
                rho9v = float(cc['rho9'][hh]); m9v = float(cc['m9_lam'][hh])
                expb = float(cc['const0_full'][hh])
                piv = [bool(cc['piv_is_h'][hh, o]) for o in range(8)]
                piv9 = bool(cc['piv9_is_B'][hh])
            P = D0ap.shape[0]
            sh = [P, BPC, NP_]
            i0, i1 = (D0ap, D1ap) if piv9 else (D1ap, D0ap)
            t9 = tpool.tile(sh, BF16, name="t9", tag="tau")
            nc.vector.scalar_tensor_tensor(t9, in0=i0, scalar=rho9v, in1=i1,
                                           op0=OP.mult, op1=OP.add)
            acc = tpool.tile(sh, BF16, name="acc", tag="acc")
            nc.vector.tensor_scalar(acc, in0=t9, scalar1=m9v, scalar2=None,
                                    op0=OP.mult)
            for o in range(8):
                i0o, i1o = (D0ap, D1ap) if piv[o] else (D1ap, D0ap)
                tau = tpool.tile(sh, BF16, name="tau", tag="tau")
                nc.vector.scalar_tensor_tensor(tau, in0=i0o, scalar=rho_o[o],
                                               in1=i1o, op0=OP.mult, op1=OP.add)
                if ragged:
                    u1 = tpool.tile(sh, BF16, name="u1", tag="u")
                    nc.vector.tensor_scalar(u1, in0=tau, scalar1=m_o[o],
                                            scalar2=d_o[o], op0=OP.mult,
                                            op1=OP.add)
                    u = tpool.tile(sh, BF16, name="u", tag="u")
                    nc.vector.scalar_tensor_tensor(u, in0=u1, scalar=-1.0,
                                                   in1=u1, op0=OP.mult,
                                                   op1=OP.max)
                    acc2 = tpool.tile(sh, BF16, name="acc2", tag="acc")
                    nc.vector.scalar_tensor_tensor(acc2, in0=u, scalar=chat_o[o],
                                                   in1=acc, op0=OP.mult,
                                                   op1=OP.add)
                else:
                    u = tpool.tile(sh, BF16, name="u", tag="u")
                    nc.vector.tensor_scalar(u, in0=tau, scalar1=M_o[o],
                                            scalar2=s2_o[o], op0=OP.mult,
                                            op1=OP.max if ismax_o[o] else OP.min)
                    acc2 = tpool.tile(sh, BF16, name="acc2", tag="acc")
                    nc.vector.tensor_add(acc2, u, acc)
                acc = acc2
            for s in range(2):
                Ds = D0ap if s == 0 else D1ap
                lg = tpool.tile(sh, BF16, name="lg", tag="u")
                nc.vector.tensor_add(lg, Ds, acc)
                nc.scalar.activation(outEap(s), lg, AF.Exp, bias=expb, scale=1.0)

        conv_unit.cur_h = None
        conv_unit(Drag_sb[:, 0, :, :], Drag_sb[:, 1, :, :],
                  lambda s: Erag_sb[:, s, :, :], ragged=True)
        # regather ragged E rows to base-partition 0 (matmul operand constraint)
        Erag2_sb = konst.tile([1, 8, 2, BPC, NP_], BF16, name="Erag2_sb")
        for hh in range(8):
            nc.sync.dma_start(out=Erag2_sb[0:1, hh], in_=Erag_sb[hh:hh + 1])

        # ---- phase 2c + 3: per head: dots, conv, attention out
        for hh in range(8):
            p0 = (hh % 2) * 64
            kft = 4 + hh // 2
            qft = hh // 2
            E_units = []
            for jt in range(2):
                D_sb = dpool.tile([128, 2, BPC, NP_], BF16, name="D_sb", tag="D")
                for b in range(BPC):
                    for s in range(2):
                        ps = ppool.tile([128, NP_], F32, name="psd", tag="ps")
                        nc.tensor.matmul(
                            ps,
                            lhsT=QK_sb[p0:p0 + 64, b, s, kft, jt * 128:(jt + 1) * 128],
                            rhs=QK_sb[p0:p0 + 64, b, s, qft, :],
                            start=True, stop=True)
                        if b < 2:
                            nc.scalar.activation(D_sb[:, s, b, :], ps, AF.Copy,
                                                 bias=0.0, scale=1.0)
                        else:
                            nc.vector.tensor_copy(D_sb[:, s, b, :], ps)
                E_sb = dpool.tile([128, 2, BPC, NP_], BF16, name="E_sb", tag="E")
                conv_unit.cur_h = hh
                conv_unit(D_sb[:, 0, :, :], D_sb[:, 1, :, :],
                          lambda s, E_sb=E_sb: E_sb[:, s, :, :], ragged=False)
                E_units.append(E_sb)
            for b in range(BPC):
                for s in range(2):
                    po = ppool.tile([65, NP_], F32, name="pso", tag="ps")
                    nc.tensor.matmul(po, lhsT=v_sb[:, b, s, 0, hh, :],
                                     rhs=E_units[0][:, s, b, :], start=True, stop=False)
                    nc.tensor.matmul(po, lhsT=v_sb[:, b, s, 1, hh, :],
                                     rhs=E_units[1][:, s, b, :], start=False, stop=False)
                    nc.tensor.matmul(po, lhsT=v_sb[0:1, b, s, 2, hh, :],
                                     rhs=Erag2_sb[0:1, hh, s, b, :], start=False,
                                     stop=True)
                    lnr = rpool.tile([1, NP_], F32, name="lnr", tag="rcp")
                    nc.scalar.activation(lnr, po[64:65, :], AF.Ln, bias=0.0,
                                         scale=1.0)
                    rcp = rpool.tile([1, NP_], F32, name="rcp", tag="rcp")
                    nc.scalar.activation(rcp, lnr, AF.Exp, bias=0.0, scale=-1.0)
                    osb = rpool.tile([64, NP_], F32, name="osb", tag="osb")
                    nc.scalar.activation(osb, po[0:64, :], AF.Copy, bias=0.0,
                                         scale=1.0)
                    rb = ppool.tile([64, NP_], F32, name="rb", tag="ps")
                    nc.tensor.matmul(rb, lhsT=onesf_sb, rhs=rcp, start=True,
                                     stop=True)
                    nc.vector.scalar_tensor_tensor(
                        outT_sb[p0:p0 + 64, b, s, hh // 2, :],
                        in0=osb, scalar=0.0, in1=rb,
                        op0=OP.bypass, op1=OP.mult)

        # ---- phase 4: MLP + store (s-outer so wm streams once per s)
        for s in range(2):
            wm_sb = xpool.tile([128, 4, 512], BF16, name="wm_sb", tag="wv", bufs=1)
            nc.sync.dma_start(out=wm_sb, in_=wm[s].rearrange("t p f -> p t f"))
            for b in range(BPC):
                for ft in range(4):
                    ps = ppool.tile([128, NP_], F32, name="psm", tag="ps")
                    for dt in range(4):
                        nc.tensor.matmul(
                            ps, lhsT=wm_sb[:, dt, ft * 128:(ft + 1) * 128],
                            rhs=outT_sb[:, b, s, dt, :], start=(dt == 0), stop=(dt == 3))
                    rs = rpool.tile([128, NP_], F32, name="rs", tag="rs")
                    nc.scalar.activation(rs, ps, AF.Identity,
                                         bias=bmv_sb[:, s, ft:ft + 1], scale=1.0)
                    nc.sync.dma_start(out=res[s, b, ft], in_=rs)

        rpool.release(); tpool.release(); dpool.release()
        xpool.release(); ppool.release(); konst.release()
    return nc


# ----------------------------------------------------------------- frontend
_cache = {}


def kernel(**inputs):
    inputs = {k: np.asarray(v) for k, v in inputs.items()}
    cc = _fold_consts(inputs)

    # host-side packing
    SC = np.float64(SCALE)
    Wq = np.asarray(inputs['Wqkv'], np.float64)
    Wq1 = np.asarray(inputs['Wqkv1'], np.float64)
    bq = np.asarray(inputs['bqkv'], np.float64)
    bq1 = np.asarray(inputs['bqkv1'], np.float64)
    Wmlp = np.asarray(inputs['Wmlp'], np.float64)
    Wmlp1 = np.asarray(inputs['Wmlp1'], np.float64)
    bmlp = np.asarray(inputs['bmlp'], np.float64)
    bmlp1 = np.asarray(inputs['bmlp1'], np.float64)

    wqk_np = np.stack([
        np.concatenate([Wq[:, 0:512] * SC, Wq[:, 512:1024]], 1),
        np.concatenate([Wq1[:, 0:512] * SC, Wq1[:, 512:1024]], 1),
    ]).reshape(2, 4, 128, 1024).astype(ml_dtypes.bfloat16)
    wv_np = np.stack([Wq[:, 1024:1536], Wq1[:, 1024:1536]]).reshape(
        2, 4, 128, 512).astype(ml_dtypes.bfloat16)
    wm_np = np.stack([Wmlp, Wmlp1]).reshape(2, 4, 128, 512).astype(ml_dtypes.bfloat16)
    qkb_np = np.stack([
        np.concatenate([bq[0:512] * SC, bq[512:1024]]),
        np.concatenate([bq1[0:512] * SC, bq1[512:1024]]),
    ]).reshape(2, 8, 128).transpose(2, 0, 1).astype(np.float32).copy()
    bm_eff = np.stack([bq[1024:1536] @ Wmlp + bmlp,
                       bq1[1024:1536] @ Wmlp1 + bmlp1])
    bmv_np = bm_eff.reshape(2, 4, 128).transpose(2, 0, 1).astype(np.float32).copy()
    cvc_np = np.concatenate([
        cc['rho_r'], cc['m_r'], cc['chat'],
        cc['rho9_r'][:, None], cc['m9_r'][:, None], cc['const0'][:, None],
    ], axis=1).astype(np.float32)  # [8, 27]
    cvc_np = np.concatenate([cvc_np, np.zeros((8, 1), np.float32)], axis=1)

    x = np.asarray(inputs['x'], np.float32)
    l = np.asarray(inputs['l'], np.float32)
    xpad = np.zeros((2, B, D, NP_), np.float32)
    xpad[0, :, :, :N] = x.transpose(0, 2, 1)
    xpad[1, :, :, :N] = l.transpose(0, 2, 1)
    xt_all = xpad.reshape(2, B, 4, 128, NP_).astype(ml_dtypes.bfloat16)

    key = hashlib.sha256()
    for nm in ('conv1_w', 'conv1_b', 'bn_g', 'bn_b', 'conv2_w', 'conv2_b'):
        key.update(np.ascontiguousarray(inputs[nm]).tobytes())
    key = key.hexdigest()
    if key not in _cache:
        _cache[key] = _build(cc)
    nc = _cache[key]

    in_maps = []
    for c in range(NCORES):
        bs = slice(c * BPC, (c + 1) * BPC)
        in_maps.append({
            "xt": np.ascontiguousarray(xt_all[:, bs]),
            "wqk": wqk_np, "wv": wv_np, "wm": wm_np,
            "qkb": qkb_np, "bmv": bmv_np, "cvc": cvc_np,
        })
    rr = run_bass_kernel_spmd(nc, in_maps, core_ids=list(range(NCORES)))
    out0 = np.empty((B, N, D), np.float32)
    out1 = np.empty((B, N, D), np.float32)
    for c in range(NCORES):
        r = rr.results[c]["res"]            # [2, BPC, 4, 128, NP_]
        r = r.reshape(2, BPC, D, NP_)[:, :, :, :N].transpose(0, 1, 3, 2)
        out0[c * BPC:(c + 1) * BPC] = r[0]
        out1[c * BPC:(c + 1) * BPC] = r[1]
    return out0, out1


# revision 13
# speedup vs baseline: 23.7137x; 23.7137x over previous
"""Bass/Tile TRN2 kernel for nn_Encoder_55233279426649 (dual-stream encoder
block with cross-stream attention-map conv).

Sharding: data-parallel over batch — 32 batches -> 8 NeuronCores x 4 batches.
Inside each core everything runs in "feature-major" (transposed) layouts so
only host-side transposes are needed. All matmuls run in bf16 on the PE
(fp32 PSUM accumulation); the attention-map conv block is decomposed as
  sup = mu*(A*D0 + B*D1 + Cd) + nu*sum_o w2_o |z_o| + c2,   mu=(1+l)/2, nu=(1-l)/2
(LeakyReLU(z) = mu*z + nu*|z|), evaluated with fused DVE ops over
batch-packed tiles. Softmax has no max-subtract (logits are O(1)); the
denominator comes free from an appended ones-column in V.
"""
import hashlib
import numpy as np
import ml_dtypes

import concourse.bass as bass
import concourse.mybir as mybir
import concourse.tile as tile
from concourse.vector_clock import ScopedClock
from concourse.bass_utils import run_bass_kernel_spmd

# ---------------------------------------------------------------- constants
B, N, D, H, NA = 32, 257, 512, 8, 8
DH = D // H
SCALE = (D / H) ** -0.5
BN_EPS = 1e-5
LAM = 0.01
MU = (1 + LAM) / 2
NU = (1 - LAM) / 2
NP_ = 258            # padded query length (even for DVE packed modes)
NCORES = 8
BPC = B // NCORES    # batches per core
F32 = mybir.dt.float32
BF16 = mybir.dt.bfloat16
AF = mybir.ActivationFunctionType
OP = mybir.AluOpType

# ------------------------------------------------- walrus 1-wait legalizer
_ctr = [0]


def _mk_wait_nop(engine, wait):
    _ctr[0] += 1
    nop = mybir.InstNoOp(name=f"Iws-{_ctr[0]}", engine=engine, ins=[], outs=[])
    nop.sync_info = mybir.SyncInfo(on_wait=[wait], on_update=[])
    return nop


class FixedTileContext(tile.TileContext):
    """Splits >1-wait instructions into wait-carrying nops (this container's
    walrus accepts at most one sync-wait command per instruction)."""

    def _lower_ordered_insts(self, postordered_blocks):
        for bb_name in list(postordered_blocks.keys()):
            insts = postordered_blocks[bb_name]
            new = []
            changed = False
            for inst in insts:
                si = inst.sync_info
                if si is not None and si.on_wait is not None and len(si.on_wait) > 1:
                    waits = list(si.on_wait)
                    for w in waits[:-1]:
                        new.append(_mk_wait_nop(inst.engine, w))
                    si.on_wait = [waits[-1]]
                    changed = True
                new.append(inst)
            if changed:
                if isinstance(insts, list):
                    insts[:] = new
                else:
                    postordered_blocks[bb_name] = new
        return super()._lower_ordered_insts(postordered_blocks)

    def _drain_and_barrier(self, tick_clock, wait_clock):
        nc = self.nc
        drain_inst = nc.sync.drain()
        wait_clock.add_sem_waits(
            drain_inst.ins, ScopedClock({None: tick_clock.global_clock})
        )
        si = drain_inst.ins.sync_info
        if si is not None and si.on_wait is not None and len(si.on_wait) > 1:
            waits = list(si.on_wait)
            si.on_wait = waits[:1]
            for w in waits[1:]:
                d2 = nc.sync.drain()
                si2 = d2.ins.sync_info
                if si2 is None:
                    d2.ins.sync_info = mybir.SyncInfo(on_wait=[w], on_update=[])
                else:
                    si2.on_wait = list(si2.on_wait or []) + [w]
        nc.all_engine_barrier()
        assert self.sems is not None
        popped = nc._tile_sem_poison_stack.pop()
        assert popped is self._sem_poison
        nc.clear_and_free_semaphores(list(self.sems.allocated().values()))
        nc.all_engine_barrier()


# ------------------------------------------------------------- host folding
def _fold_consts(inputs):
    """Returns dict of host-folded constants (f64 where it matters)."""
    conv1_w = np.asarray(inputs['conv1_w'], np.float64)
    conv1_b = np.asarray(inputs['conv1_b'], np.float64)
    bn_g = np.asarray(inputs['bn_g'], np.float64)
    bn_b = np.asarray(inputs['bn_b'], np.float64)
    conv2_w = np.asarray(inputs['conv2_w'], np.float64)
    conv2_b = np.asarray(inputs['conv2_b'], np.float64)
    inv = 1.0 / np.sqrt(1.0 + BN_EPS)
    g = conv1_w[:, :, 0] * bn_g * inv          # [H, NA] coef on D0 (dots)
    h = conv1_w[:, :, 1] * bn_g * inv          # coef on D1 (dots1)
    d = conv1_b * bn_g * inv + bn_b            # [H, NA]
    w2 = conv2_w
    A = (w2 * g).sum(1)
    Bc = (w2 * h).sum(1)
    Cd = (w2 * d).sum(1)
    const0 = MU * Cd + conv2_b                 # exp bias per head
    eps = 1e-30
    piv_is_h = np.abs(h) >= np.abs(g)
    hs = np.where(np.abs(h) < eps, eps, h)
    gs = np.where(np.abs(g) < eps, eps, g)
    rho = np.where(piv_is_h, g / hs, h / gs)
    m = np.where(piv_is_h, hs, gs)
    chat = NU * w2
    As = np.where(np.abs(A) < eps, eps, A)
    Bs = np.where(np.abs(Bc) < eps, eps, Bc)
    piv9_is_B = np.abs(Bc) >= np.abs(A)
    rho9 = np.where(piv9_is_B, A / Bs, Bc / As)
    m9 = np.where(piv9_is_B, Bs, As) * MU
    # ragged (fixed pivot = h / B)
    rho_r = g / hs
    m_r = hs
    rho9_r = A / Bs
    m9_r = Bs * MU
    # full units use the relu form: LReLU(z) = lam*z + (1-lam)*relu(z)
    c_full = (1 - LAM) * w2                       # [H, NA]
    M_full = c_full * m                           # scalar1 for opB
    s2_full = -c_full * d                         # scalar2 for opB (max/min shift)
    is_max = w2 >= 0
    m9_lam = np.where(piv9_is_B, Bs, As) * LAM
    const0_full = LAM * Cd + conv2_b + (c_full * d).sum(1)
    return dict(g=g, h=h, d=d, piv_is_h=piv_is_h, rho=rho, m=m, chat=chat,
                rho9=rho9, m9=m9, piv9_is_B=piv9_is_B, const0=const0,
                rho_r=rho_r, m_r=m_r, rho9_r=rho9_r, m9_r=m9_r,
                M_full=M_full, s2_full=s2_full, is_max=is_max,
                m9_lam=m9_lam, const0_full=const0_full)


# ------------------------------------------------------------- bass builder
def _build(cc):
    """cc: dict of folded conv consts (floats embedded as immediates)."""
    nc = bass.Bass()
    xt = nc.dram_tensor("xt", [2, BPC, 4, 128, NP_], BF16, kind="ExternalInput")
    wqk = nc.dram_tensor("wqk", [2, 4, 128, 1024], BF16, kind="ExternalInput")
    wv = nc.dram_tensor("wv", [2, 4, 128, 512], BF16, kind="ExternalInput")
    wm = nc.dram_tensor("wm", [2, 4, 128, 512], BF16, kind="ExternalInput")
    qkb = nc.dram_tensor("qkb", [128, 2, 8], F32, kind="ExternalInput")
    bmv = nc.dram_tensor("bmv", [128, 2, 4], F32, kind="ExternalInput")
    cvc = nc.dram_tensor("cvc", [8, 28], F32, kind="ExternalInput")
    res = nc.dram_tensor("res", [2, BPC, 4, 128, NP_], F32, kind="ExternalOutput")

    with FixedTileContext(nc) as tc:
        konst = tc.alloc_tile_pool(name="konst", bufs=1)
        ppool = tc.alloc_tile_pool(name="ppool", bufs=8, space="PSUM")
        xpool = tc.alloc_tile_pool(name="xpool", bufs=2)
        dpool = tc.alloc_tile_pool(name="dpool", bufs=2)
        tpool = tc.alloc_tile_pool(name="tpool", bufs=2)
        rpool = tc.alloc_tile_pool(name="rpool", bufs=2)

        # ---- resident constants/weights
        qkb_sb = konst.tile([128, 2, 8], F32, name="qkb_sb")
        nc.sync.dma_start(out=qkb_sb, in_=qkb[:, :, :])
        bmv_sb = konst.tile([128, 2, 4], F32, name="bmv_sb")
        nc.sync.dma_start(out=bmv_sb, in_=bmv[:, :, :])
        cvc_sb = konst.tile([8, 28], F32, name="cvc_sb")
        nc.sync.dma_start(out=cvc_sb, in_=cvc[:, :])
        ones_sb = konst.tile([1, 64], BF16, name="ones_sb")
        nc.vector.memset(ones_sb, 1.0)
        onesf_sb = konst.tile([1, 64], F32, name="onesf_sb")
        nc.vector.memset(onesf_sb, 1.0)

        QK_sb = konst.tile([128, BPC, 2, 8, NP_], BF16, name="QK_sb")
        v_sb = konst.tile([128, BPC, 2, 3, 8, 65], BF16, name="v_sb")
        outT_sb = konst.tile([128, BPC, 2, 4, NP_], BF16, name="outT_sb")
        rag_sb = konst.tile([40, 2, BPC, NP_], BF16, name="rag_sb")
        Drag_sb = rag_sb[0:8]
        Erag_sb = rag_sb[32:40]

        # ---- phase 1: projections (s-outer so weights stream once per s)
        for s in range(2):
            wqk_sb = xpool.tile([128, 4, 1024], BF16, name="wqk_sb", tag="wqk", bufs=1)
            nc.sync.dma_start(out=wqk_sb, in_=wqk[s].rearrange("t p f -> p t f"))
            wv_sb = xpool.tile([128, 4, 512], BF16, name="wv_sb", tag="wv", bufs=1)
            nc.sync.dma_start(out=wv_sb, in_=wv[s].rearrange("t p f -> p t f"))
            for b in range(BPC):
                xts = xpool.tile([128, 4, NP_], BF16, name="xts", tag="xts")
                nc.sync.dma_start(out=xts, in_=xt[s, b].rearrange("t p i -> p t i"))
                for ft in range(8):
                    ps = ppool.tile([128, NP_], F32, name="psqk", tag="ps")
                    for dt in range(4):
                        nc.tensor.matmul(
                            ps, lhsT=wqk_sb[:, dt, ft * 128:(ft + 1) * 128],
                            rhs=xts[:, dt, :], start=(dt == 0), stop=(dt == 3))
                    nc.scalar.activation(QK_sb[:, b, s, ft, :], ps, AF.Identity,
                                         bias=qkb_sb[:, s, ft:ft + 1], scale=1.0)
                for tt in range(3):
                    tsz = 128 if tt < 2 else 1
                    ps = ppool.tile([128, 512], F32, name="psv", tag="ps")
                    for dt in range(4):
                        nc.tensor.matmul(
                            ps[0:tsz, :], lhsT=xts[:, dt, tt * 128:tt * 128 + tsz],
                            rhs=wv_sb[:, dt, :], start=(dt == 0), stop=(dt == 3))
                    nc.vector.tensor_copy(v_sb[0:tsz, b, s, tt, :, 0:64],
                                          ps[0:tsz, :].rearrange("p (h e) -> p h e", h=8))
                    nc.vector.memset(v_sb[0:tsz, b, s, tt, :, 64:65], 1.0)

        # ---- phase 2a: ragged (j=256) dots for all heads -> Drag
        Dragf_sb = konst.tile([1, 8, 2, BPC, NP_], BF16, name="Dragf_sb")
        for b in range(BPC):
            for s in range(2):
                for hh in range(8):
                    p0 = (hh % 2) * 64
                    kft = 4 + hh // 2
                    qft = hh // 2
                    psr = ppool.tile([1, NP_], F32, name="psr", tag="ps")
                    nc.tensor.matmul(
                        psr,
                        lhsT=QK_sb[p0:p0 + 64, b, s, kft, 256:257],
                        rhs=QK_sb[p0:p0 + 64, b, s, qft, :],
                        start=True, stop=True)
                    nc.scalar.activation(Dragf_sb[0:1, hh, s, b, :], psr, AF.Copy,
                                         bias=0.0, scale=1.0)
        for hh in range(8):
            nc.sync.dma_start(out=Drag_sb[hh:hh + 1], in_=Dragf_sb[0:1, hh])

        # ---- phase 2b: ragged conv unit (per-partition consts over heads)
        def conv_unit(D0ap, D1ap, outEap, ragged):
            """D0ap/D1ap: [P, BPC, NP_] bf16 views. outEap(s) -> E view."""
            if ragged:
                def sc(col):
                    return cvc_sb[:, col:col + 1]
                rho_o = [sc(o) for o in range(8)]
                m_o = [sc(8 + o) for o in range(8)]
                chat_o = [sc(16 + o) for o in range(8)]
                rho9v, m9v, expb = sc(24), sc(25), sc(26)
                d_o = [sc(27) for o in range(8)]
                piv = [True] * 8
                piv9 = True
            else:
                hh = conv_unit.cur_h
                rho_o = [float(cc['rho'][hh, o]) for o in range(8)]
                M_o = [float(cc['M_full'][hh, o]) for o in range(8)]
                s2_o = [float(cc['s2_full'][hh, o]) for o in range(8)]
                ismax_o = [bool(cc['is_max'][hh, o]) for o in range(8)]
                rho9v = float(cc['rho9'][hh]); m9v = float(cc['m9_lam'][hh])
                expb = float(cc['const0_full'][hh])
                piv = [bool(cc['piv_is_h'][hh, o]) for o in range(8)]
                piv9 = bool(cc['piv9_is_B'][hh])
            P = D0ap.shape[0]
            sh = [P, BPC, NP_]
            i0, i1 = (D0ap, D1ap) if piv9 else (D1ap, D0ap)
            t9 = tpool.tile(sh, BF16, name="t9", tag="tau")
            nc.vector.scalar_tensor_tensor(t9, in0=i0, scalar=rho9v, in1=i1,
                                           op0=OP.mult, op1=OP.add)
            acc = tpool.tile(sh, BF16, name="acc", tag="acc")
            nc.vector.tensor_scalar(acc, in0=t9, scalar1=m9v, scalar2=None,
                                    op0=OP.mult)
            for o in range(8):
                i0o, i1o = (D0ap, D1ap) if piv[o] else (D1ap, D0ap)
                tau = tpool.tile(sh, BF16, name="tau", tag="tau")
                nc.vector.scalar_tensor_tensor(tau, in0=i0o, scalar=rho_o[o],
                                               in1=i1o, op0=OP.mult, op1=OP.add)
                if ragged:
                    u1 = tpool.tile(sh, BF16, name="u1", tag="u")
                    nc.vector.tensor_scalar(u1, in0=tau, scalar1=m_o[o],
                                            scalar2=d_o[o], op0=OP.mult,
                                            op1=OP.add)
                    u = tpool.tile(sh, BF16, name="u", tag="u")
                    nc.vector.scalar_tensor_tensor(u, in0=u1, scalar=-1.0,
                                                   in1=u1, op0=OP.mult,
                                                   op1=OP.max)
                    acc2 = tpool.tile(sh, BF16, name="acc2", tag="acc")
                    nc.vector.scalar_tensor_tensor(acc2, in0=u, scalar=chat_o[o],
                                                   in1=acc, op0=OP.mult,
                                                   op1=OP.add)
                else:
                    u = tpool.tile(sh, BF16, name="u", tag="u")
                    nc.vector.tensor_scalar(u, in0=tau, scalar1=M_o[o],
                                            scalar2=s2_o[o], op0=OP.mult,
                                            op1=OP.max if ismax_o[o] else OP.min)
                    acc2 = tpool.tile(sh, BF16, name="acc2", tag="acc")
                    nc.vector.tensor_add(acc2, u, acc)
                acc = acc2
            for s in range(2):
                Ds = D0ap if s == 0 else D1ap
                lg = tpool.tile(sh, BF16, name="lg", tag="u")
                nc.vector.tensor_add(lg, Ds, acc)
                nc.scalar.activation(outEap(s), lg, AF.Exp, bias=expb, scale=1.0)

        conv_unit.cur_h = None
        conv_unit(Drag_sb[:, 0, :, :], Drag_sb[:, 1, :, :],
                  lambda s: Erag_sb[:, s, :, :], ragged=True)
        # regather ragged E rows to base-partition 0 (matmul operand constraint)
        Erag2_sb = konst.tile([1, 8, 2, BPC, NP_], BF16, name="Erag2_sb")
        for hh in range(8):
            nc.sync.dma_start(out=Erag2_sb[0:1, hh], in_=Erag_sb[hh:hh + 1])

        # ---- phase 2c + 3: per head: dots, conv, attention out
        for hh in range(8):
            p0 = (hh % 2) * 64
            kft = 4 + hh // 2
            qft = hh // 2
            E_units = []
            for jt in range(2):
                D_sb = dpool.tile([128, 2, BPC, NP_], BF16, name="D_sb", tag="D")
                for b in range(BPC):
                    for s in range(2):
                        ps = ppool.tile([128, NP_], F32, name="psd", tag="ps")
                        nc.tensor.matmul(
                            ps,
                            lhsT=QK_sb[p0:p0 + 64, b, s, kft, jt * 128:(jt + 1) * 128],
                            rhs=QK_sb[p0:p0 + 64, b, s, qft, :],
                            start=True, stop=True)
                        if b < 2:
                            nc.scalar.activation(D_sb[:, s, b, :], ps, AF.Copy,
                                                 bias=0.0, scale=1.0)
                        else:
                            nc.vector.tensor_copy(D_sb[:, s, b, :], ps)
                E_sb = dpool.tile([128, 2, BPC, NP_], BF16, name="E_sb", tag="E")
                conv_unit.cur_h = hh
                conv_unit(D_sb[:, 0, :, :], D_sb[:, 1, :, :],
                          lambda s, E_sb=E_sb: E_sb[:, s, :, :], ragged=False)
                E_units.append(E_sb)
            for b in range(BPC):
                for s in range(2):
                    po = ppool.tile([65, NP_], F32, name="pso", tag="ps")
                    nc.tensor.matmul(po, lhsT=v_sb[:, b, s, 0, hh, :],
                                     rhs=E_units[0][:, s, b, :], start=True, stop=False)
                    nc.tensor.matmul(po, lhsT=v_sb[:, b, s, 1, hh, :],
                                     rhs=E_units[1][:, s, b, :], start=False, stop=False)
                    nc.tensor.matmul(po, lhsT=v_sb[0:1, b, s, 2, hh, :],
                                     rhs=Erag2_sb[0:1, hh, s, b, :], start=False,
                                     stop=True)
                    lnr = rpool.tile([1, NP_], F32, name="lnr", tag="rcp")
                    nc.scalar.activation(lnr, po[64:65, :], AF.Ln, bias=0.0,
                                         scale=1.0)
                    rcp = rpool.tile([1, NP_], F32, name="rcp", tag="rcp")
                    nc.scalar.activation(rcp, lnr, AF.Exp, bias=0.0, scale=-1.0)
                    osb = rpool.tile([64, NP_], F32, name="osb", tag="osb")
                    nc.scalar.activation(osb, po[0:64, :], AF.Copy, bias=0.0,
                                         scale=1.0)
                    rb = ppool.tile([64, NP_], F32, name="rb", tag="ps")
                    nc.tensor.matmul(rb, lhsT=onesf_sb, rhs=rcp, start=True,
                                     stop=True)
                    nc.vector.scalar_tensor_tensor(
                        outT_sb[p0:p0 + 64, b, s, hh // 2, :],
                        in0=osb, scalar=0.0, in1=rb,
                        op0=OP.bypass, op1=OP.mult)

        # ---- phase 4: MLP + store (s-outer so wm streams once per s)
        for s in range(2):
            wm_sb = xpool.tile([128, 4, 512], BF16, name="wm_sb", tag="wv", bufs=1)
            nc.sync.dma_start(out=wm_sb, in_=wm[s].rearrange("t p f -> p t f"))
            for b in range(BPC):
                for ft in range(4):
                    ps = ppool.tile([128, NP_], F32, name="psm", tag="ps")
                    for dt in range(4):
                        nc.tensor.matmul(
                            ps, lhsT=wm_sb[:, dt, ft * 128:(ft + 1) * 128],
                            rhs=outT_sb[:, b, s, dt, :], start=(dt == 0), stop=(dt == 3))
                    rs = rpool.tile([128, NP_], F32, name="rs", tag="rs")
                    nc.scalar.activation(rs, ps, AF.Identity,
                                         bias=bmv_sb[:, s, ft:ft + 1], scale=1.0)
                    nc.sync.dma_start(out=res[s, b, ft], in_=rs)

        rpool.release(); tpool.release(); dpool.release()
        xpool.release(); ppool.release(); konst.release()
    return nc


# ----------------------------------------------------------------- frontend
_cache = {}


def kernel(**inputs):
    inputs = {k: np.asarray(v) for k, v in inputs.items()}
    cc = _fold_consts(inputs)

    # host-side packing
    SC = np.float64(SCALE)
    Wq = np.asarray(inputs['Wqkv'], np.float64)
    Wq1 = np.asarray(inputs['Wqkv1'], np.float64)
    bq = np.asarray(inputs['bqkv'], np.float64)
    bq1 = np.asarray(inputs['bqkv1'], np.float64)
    Wmlp = np.asarray(inputs['Wmlp'], np.float64)
    Wmlp1 = np.asarray(inputs['Wmlp1'], np.float64)
    bmlp = np.asarray(inputs['bmlp'], np.float64)
    bmlp1 = np.asarray(inputs['bmlp1'], np.float64)

    wqk_np = np.stack([
        np.concatenate([Wq[:, 0:512] * SC, Wq[:, 512:1024]], 1),
        np.concatenate([Wq1[:, 0:512] * SC, Wq1[:, 512:1024]], 1),
    ]).reshape(2, 4, 128, 1024).astype(ml_dtypes.bfloat16)
    wv_np = np.stack([Wq[:, 1024:1536], Wq1[:, 1024:1536]]).reshape(
        2, 4, 128, 512).astype(ml_dtypes.bfloat16)
    wm_np = np.stack([Wmlp, Wmlp1]).reshape(2, 4, 128, 512).astype(ml_dtypes.bfloat16)
    qkb_np = np.stack([
        np.concatenate([bq[0:512] * SC, bq[512:1024]]),
        np.concatenate([bq1[0:512] * SC, bq1[512:1024]]),
    ]).reshape(2, 8, 128).transpose(2, 0, 1).astype(np.float32).copy()
    bm_eff = np.stack([bq[1024:1536] @ Wmlp + bmlp,
                       bq1[1024:1536] @ Wmlp1 + bmlp1])
    bmv_np = bm_eff.reshape(2, 4, 128).transpose(2, 0, 1).astype(np.float32).copy()
    cvc_np = np.concatenate([
        cc['rho_r'], cc['m_r'], cc['chat'],
        cc['rho9_r'][:, None], cc['m9_r'][:, None], cc['const0'][:, None],
    ], axis=1).astype(np.float32)  # [8, 27]
    cvc_np = np.concatenate([cvc_np, np.zeros((8, 1), np.float32)], axis=1)

    x = np.asarray(inputs['x'], np.float32)
    l = np.asarray(inputs['l'], np.float32)
    xpad = np.zeros((2, B, D, NP_), np.float32)
    xpad[0, :, :, :N] = x.transpose(0, 2, 1)
    xpad[1, :, :, :N] = l.transpose(0, 2, 1)
    xt_all = xpad.reshape(2, B, 4, 128, NP_).astype(ml_dtypes.bfloat16)

    key = hashlib.sha256()
    for nm in ('conv1_w', 'conv1_b', 'bn_g', 'bn_b', 'conv2_w', 'conv2_b'):
        key.update(np.ascontiguousarray(inputs[nm]).tobytes())
    key = key.hexdigest()
    if key not in _cache:
        _cache[key] = _build(cc)
    nc = _cache[key]

    in_maps = []
    for c in range(NCORES):
        bs = slice(c * BPC, (c + 1) * BPC)
        in_maps.append({
            "xt": np.ascontiguousarray(xt_all[:, bs]),
            "wqk": wqk_np, "wv": wv_np, "wm": wm_np,
            "qkb": qkb_np, "bmv": bmv_np, "cvc": cvc_np,
        })
    globals()['_last_in_maps'] = in_maps
    rr = run_bass_kernel_spmd(nc, in_maps, core_ids=list(range(NCORES)))
    out0 = np.empty((B, N, D), np.float32)
    out1 = np.empty((B, N, D), np.float32)
    for c in range(NCORES):
        r = rr.results[c]["res"]            # [2, BPC, 4, 128, NP_]
        r = r.reshape(2, BPC, D, NP_)[:, :, :, :N].transpose(0, 1, 3, 2)
        out0[c * BPC:(c + 1) * BPC] = r[0]
        out1[c * BPC:(c + 1) * BPC] = r[1]
    return out0, out1


# revision 15
# speedup vs baseline: 24.9659x; 1.0528x over previous
"""Bass/Tile TRN2 kernel for nn_Encoder_55233279426649 (dual-stream encoder
block with cross-stream attention-map conv).

Sharding: data-parallel over batch — 32 batches -> 8 NeuronCores x 4 batches.
Inside each core everything runs in "feature-major" (transposed) layouts so
only host-side transposes are needed. All matmuls run in bf16 on the PE
(fp32 PSUM accumulation); the attention-map conv block is decomposed as
  sup = mu*(A*D0 + B*D1 + Cd) + nu*sum_o w2_o |z_o| + c2,   mu=(1+l)/2, nu=(1-l)/2
(LeakyReLU(z) = mu*z + nu*|z|), evaluated with fused DVE ops over
batch-packed tiles. Softmax has no max-subtract (logits are O(1)); the
denominator comes free from an appended ones-column in V.
"""
import hashlib
import numpy as np
import ml_dtypes

import concourse.bass as bass
import concourse.mybir as mybir
import concourse.tile as tile
from concourse.vector_clock import ScopedClock
from concourse.bass_utils import run_bass_kernel_spmd

# ---------------------------------------------------------------- constants
B, N, D, H, NA = 32, 257, 512, 8, 8
DH = D // H
SCALE = (D / H) ** -0.5
BN_EPS = 1e-5
LAM = 0.01
MU = (1 + LAM) / 2
NU = (1 - LAM) / 2
NP_ = 258            # padded query length (even for DVE packed modes)
NCORES = 8
BPC = B // NCORES    # batches per core
F32 = mybir.dt.float32
BF16 = mybir.dt.bfloat16
AF = mybir.ActivationFunctionType
OP = mybir.AluOpType

# ------------------------------------------------- walrus 1-wait legalizer
_ctr = [0]


def _mk_wait_nop(engine, wait):
    _ctr[0] += 1
    nop = mybir.InstNoOp(name=f"Iws-{_ctr[0]}", engine=engine, ins=[], outs=[])
    nop.sync_info = mybir.SyncInfo(on_wait=[wait], on_update=[])
    return nop


class FixedTileContext(tile.TileContext):
    """Splits >1-wait instructions into wait-carrying nops (this container's
    walrus accepts at most one sync-wait command per instruction)."""

    def _lower_ordered_insts(self, postordered_blocks):
        for bb_name in list(postordered_blocks.keys()):
            insts = postordered_blocks[bb_name]
            new = []
            changed = False
            for inst in insts:
                si = inst.sync_info
                if si is not None and si.on_wait is not None and len(si.on_wait) > 1:
                    waits = list(si.on_wait)
                    for w in waits[:-1]:
                        new.append(_mk_wait_nop(inst.engine, w))
                    si.on_wait = [waits[-1]]
                    changed = True
                new.append(inst)
            if changed:
                if isinstance(insts, list):
                    insts[:] = new
                else:
                    postordered_blocks[bb_name] = new
        return super()._lower_ordered_insts(postordered_blocks)

    def _drain_and_barrier(self, tick_clock, wait_clock):
        nc = self.nc
        drain_inst = nc.sync.drain()
        wait_clock.add_sem_waits(
            drain_inst.ins, ScopedClock({None: tick_clock.global_clock})
        )
        si = drain_inst.ins.sync_info
        if si is not None and si.on_wait is not None and len(si.on_wait) > 1:
            waits = list(si.on_wait)
            si.on_wait = waits[:1]
            for w in waits[1:]:
                d2 = nc.sync.drain()
                si2 = d2.ins.sync_info
                if si2 is None:
                    d2.ins.sync_info = mybir.SyncInfo(on_wait=[w], on_update=[])
                else:
                    si2.on_wait = list(si2.on_wait or []) + [w]
        nc.all_engine_barrier()
        assert self.sems is not None
        popped = nc._tile_sem_poison_stack.pop()
        assert popped is self._sem_poison
        nc.clear_and_free_semaphores(list(self.sems.allocated().values()))
        nc.all_engine_barrier()


# ------------------------------------------------------------- host folding
def _fold_consts(inputs):
    """Returns dict of host-folded constants (f64 where it matters)."""
    conv1_w = np.asarray(inputs['conv1_w'], np.float64)
    conv1_b = np.asarray(inputs['conv1_b'], np.float64)
    bn_g = np.asarray(inputs['bn_g'], np.float64)
    bn_b = np.asarray(inputs['bn_b'], np.float64)
    conv2_w = np.asarray(inputs['conv2_w'], np.float64)
    conv2_b = np.asarray(inputs['conv2_b'], np.float64)
    inv = 1.0 / np.sqrt(1.0 + BN_EPS)
    g = conv1_w[:, :, 0] * bn_g * inv          # [H, NA] coef on D0 (dots)
    h = conv1_w[:, :, 1] * bn_g * inv          # coef on D1 (dots1)
    d = conv1_b * bn_g * inv + bn_b            # [H, NA]
    w2 = conv2_w
    A = (w2 * g).sum(1)
    Bc = (w2 * h).sum(1)
    Cd = (w2 * d).sum(1)
    const0 = MU * Cd + conv2_b                 # exp bias per head
    eps = 1e-30
    piv_is_h = np.abs(h) >= np.abs(g)
    hs = np.where(np.abs(h) < eps, eps, h)
    gs = np.where(np.abs(g) < eps, eps, g)
    rho = np.where(piv_is_h, g / hs, h / gs)
    m = np.where(piv_is_h, hs, gs)
    chat = NU * w2
    As = np.where(np.abs(A) < eps, eps, A)
    Bs = np.where(np.abs(Bc) < eps, eps, Bc)
    piv9_is_B = np.abs(Bc) >= np.abs(A)
    rho9 = np.where(piv9_is_B, A / Bs, Bc / As)
    m9 = np.where(piv9_is_B, Bs, As) * MU
    # ragged (fixed pivot = h / B)
    rho_r = g / hs
    m_r = hs
    rho9_r = A / Bs
    m9_r = Bs * MU
    # full units use the relu form: LReLU(z) = lam*z + (1-lam)*relu(z)
    c_full = (1 - LAM) * w2                       # [H, NA]
    M_full = c_full * m                           # scalar1 for opB
    s2_full = -c_full * d                         # scalar2 for opB (max/min shift)
    is_max = w2 >= 0
    m9_lam = np.where(piv9_is_B, Bs, As) * LAM
    const0_full = LAM * Cd + conv2_b + (c_full * d).sum(1)
    return dict(g=g, h=h, d=d, piv_is_h=piv_is_h, rho=rho, m=m, chat=chat,
                rho9=rho9, m9=m9, piv9_is_B=piv9_is_B, const0=const0,
                rho_r=rho_r, m_r=m_r, rho9_r=rho9_r, m9_r=m9_r,
                M_full=M_full, s2_full=s2_full, is_max=is_max,
                m9_lam=m9_lam, const0_full=const0_full)


# ------------------------------------------------------------- bass builder
def _build(cc):
    """cc: dict of folded conv consts (floats embedded as immediates)."""
    nc = bass.Bass()
    xt = nc.dram_tensor("xt", [2, BPC, 4, 128, NP_], BF16, kind="ExternalInput")
    wqk = nc.dram_tensor("wqk", [2, 4, 128, 1024], BF16, kind="ExternalInput")
    wv = nc.dram_tensor("wv", [2, 4, 128, 512], BF16, kind="ExternalInput")
    wm = nc.dram_tensor("wm", [2, 4, 128, 512], BF16, kind="ExternalInput")
    qkb = nc.dram_tensor("qkb", [128, 2, 8], F32, kind="ExternalInput")
    bmv = nc.dram_tensor("bmv", [128, 2, 4], F32, kind="ExternalInput")
    cvc = nc.dram_tensor("cvc", [8, 28], F32, kind="ExternalInput")
    res = nc.dram_tensor("res", [2, BPC, 4, 128, NP_], F32, kind="ExternalOutput")

    with FixedTileContext(nc) as tc:
        konst = tc.alloc_tile_pool(name="konst", bufs=1)
        ppool = tc.alloc_tile_pool(name="ppool", bufs=8, space="PSUM")
        xpool = tc.alloc_tile_pool(name="xpool", bufs=2)
        dpool = tc.alloc_tile_pool(name="dpool", bufs=2)
        tpool = tc.alloc_tile_pool(name="tpool", bufs=2)
        rpool = tc.alloc_tile_pool(name="rpool", bufs=2)

        # ---- resident constants/weights
        qkb_sb = konst.tile([128, 2, 8], F32, name="qkb_sb")
        nc.sync.dma_start(out=qkb_sb, in_=qkb[:, :, :])
        bmv_sb = konst.tile([128, 2, 4], F32, name="bmv_sb")
        nc.sync.dma_start(out=bmv_sb, in_=bmv[:, :, :])
        cvc_sb = konst.tile([8, 28], F32, name="cvc_sb")
        nc.sync.dma_start(out=cvc_sb, in_=cvc[:, :])
        ones_sb = konst.tile([1, 64], BF16, name="ones_sb")
        nc.vector.memset(ones_sb, 1.0)
        onesf_sb = konst.tile([1, 64], F32, name="onesf_sb")
        nc.vector.memset(onesf_sb, 1.0)

        QK_sb = konst.tile([128, BPC, 2, 8, NP_], BF16, name="QK_sb")
        v_sb = konst.tile([128, BPC, 2, 3, 8, 65], BF16, name="v_sb")
        outT_sb = konst.tile([128, BPC, 2, 4, NP_], BF16, name="outT_sb")
        rag_sb = konst.tile([40, 2, BPC, NP_], BF16, name="rag_sb")
        Drag_sb = rag_sb[0:8]
        Erag_sb = rag_sb[32:40]

        # ---- phase 1: projections (s-outer so weights stream once per s)
        for s in range(2):
            wqk_sb = xpool.tile([128, 4, 1024], BF16, name="wqk_sb", tag="wqk", bufs=1)
            nc.sync.dma_start(out=wqk_sb, in_=wqk[s].rearrange("t p f -> p t f"))
            wv_sb = xpool.tile([128, 4, 512], BF16, name="wv_sb", tag="wv", bufs=1)
            nc.sync.dma_start(out=wv_sb, in_=wv[s].rearrange("t p f -> p t f"))
            for b in range(BPC):
                xts = xpool.tile([128, 4, NP_], BF16, name="xts", tag="xts")
                nc.sync.dma_start(out=xts, in_=xt[s, b].rearrange("t p i -> p t i"))
                for ft in range(8):
                    ps = ppool.tile([128, NP_], F32, name="psqk", tag="ps")
                    for dt in range(4):
                        nc.tensor.matmul(
                            ps, lhsT=wqk_sb[:, dt, ft * 128:(ft + 1) * 128],
                            rhs=xts[:, dt, :], start=(dt == 0), stop=(dt == 3))
                    nc.scalar.activation(QK_sb[:, b, s, ft, :], ps, AF.Identity,
                                         bias=qkb_sb[:, s, ft:ft + 1], scale=1.0)
                for tt in range(3):
                    tsz = 128 if tt < 2 else 1
                    ps = ppool.tile([128, 512], F32, name="psv", tag="ps")
                    for dt in range(4):
                        nc.tensor.matmul(
                            ps[0:tsz, :], lhsT=xts[:, dt, tt * 128:tt * 128 + tsz],
                            rhs=wv_sb[:, dt, :], start=(dt == 0), stop=(dt == 3))
                    nc.scalar.activation(
                        v_sb[0:tsz, b, s, tt, :, 0:64],
                        ps[0:tsz, :].rearrange("p (h e) -> p h e", h=8),
                        AF.Copy, bias=0.0, scale=1.0)
                    nc.vector.memset(v_sb[0:tsz, b, s, tt, :, 64:65], 1.0)

        # ---- phase 2a: ragged (j=256) dots for all heads -> Drag
        Dragf_sb = konst.tile([1, 8, 2, BPC, NP_], BF16, name="Dragf_sb")
        for b in range(BPC):
            for s in range(2):
                for hh in range(8):
                    p0 = (hh % 2) * 64
                    kft = 4 + hh // 2
                    qft = hh // 2
                    psr = ppool.tile([1, NP_], F32, name="psr", tag="ps")
                    nc.tensor.matmul(
                        psr,
                        lhsT=QK_sb[p0:p0 + 64, b, s, kft, 256:257],
                        rhs=QK_sb[p0:p0 + 64, b, s, qft, :],
                        start=True, stop=True)
                    nc.scalar.activation(Dragf_sb[0:1, hh, s, b, :], psr, AF.Copy,
                                         bias=0.0, scale=1.0)
        for hh in range(8):
            nc.sync.dma_start(out=Drag_sb[hh:hh + 1], in_=Dragf_sb[0:1, hh])

        # ---- phase 2b: ragged conv unit (per-partition consts over heads)
        def conv_unit(D0ap, D1ap, outEap, ragged):
            """D0ap/D1ap: [P, BPC, NP_] bf16 views. outEap(s) -> E view."""
            if ragged:
                def sc(col):
                    return cvc_sb[:, col:col + 1]
                rho_o = [sc(o) for o in range(8)]
                m_o = [sc(8 + o) for o in range(8)]
                chat_o = [sc(16 + o) for o in range(8)]
                rho9v, m9v, expb = sc(24), sc(25), sc(26)
                d_o = [sc(27) for o in range(8)]
                piv = [True] * 8
                piv9 = True
            else:
                hh = conv_unit.cur_h
                rho_o = [float(cc['rho'][hh, o]) for o in range(8)]
                M_o = [float(cc['M_full'][hh, o]) for o in range(8)]
                s2_o = [float(cc['s2_full'][hh, o]) for o in range(8)]
                ismax_o = [bool(cc['is_max'][hh, o]) for o in range(8)]
                rho9v = float(cc['rho9'][hh]); m9v = float(cc['m9_lam'][hh])
                expb = float(cc['const0_full'][hh])
                piv = [bool(cc['piv_is_h'][hh, o]) for o in range(8)]
                piv9 = bool(cc['piv9_is_B'][hh])
            P = D0ap.shape[0]
            sh = [P, BPC, NP_]
            i0, i1 = (D0ap, D1ap) if piv9 else (D1ap, D0ap)
            t9 = tpool.tile(sh, BF16, name="t9", tag="tau")
            nc.vector.scalar_tensor_tensor(t9, in0=i0, scalar=rho9v, in1=i1,
                                           op0=OP.mult, op1=OP.add)
            acc = tpool.tile(sh, BF16, name="acc", tag="acc")
            nc.vector.tensor_scalar(acc, in0=t9, scalar1=m9v, scalar2=None,
                                    op0=OP.mult)
            for o in range(8):
                i0o, i1o = (D0ap, D1ap) if piv[o] else (D1ap, D0ap)
                tau = tpool.tile(sh, BF16, name="tau", tag="tau")
                nc.vector.scalar_tensor_tensor(tau, in0=i0o, scalar=rho_o[o],
                                               in1=i1o, op0=OP.mult, op1=OP.add)
                if ragged:
                    u1 = tpool.tile(sh, BF16, name="u1", tag="u")
                    nc.vector.tensor_scalar(u1, in0=tau, scalar1=m_o[o],
                                            scalar2=d_o[o], op0=OP.mult,
                                            op1=OP.add)
                    u = tpool.tile(sh, BF16, name="u", tag="u")
                    nc.vector.scalar_tensor_tensor(u, in0=u1, scalar=-1.0,
                                                   in1=u1, op0=OP.mult,
                                                   op1=OP.max)
                    acc2 = tpool.tile(sh, BF16, name="acc2", tag="acc")
                    nc.vector.scalar_tensor_tensor(acc2, in0=u, scalar=chat_o[o],
                                                   in1=acc, op0=OP.mult,
                                                   op1=OP.add)
                else:
                    u = tpool.tile(sh, BF16, name="u", tag="u")
                    nc.vector.tensor_scalar(u, in0=tau, scalar1=M_o[o],
                                            scalar2=s2_o[o], op0=OP.mult,
                                            op1=OP.max if ismax_o[o] else OP.min)
                    acc2 = tpool.tile(sh, BF16, name="acc2", tag="acc")
                    nc.vector.tensor_add(acc2, u, acc)
                acc = acc2
            for s in range(2):
                Ds = D0ap if s == 0 else D1ap
                lg = tpool.tile(sh, BF16, name="lg", tag="u")
                nc.vector.tensor_add(lg, Ds, acc)
                nc.scalar.activation(outEap(s), lg, AF.Exp, bias=expb, scale=1.0)

        conv_unit.cur_h = None
        conv_unit(Drag_sb[:, 0, :, :], Drag_sb[:, 1, :, :],
                  lambda s: Erag_sb[:, s, :, :], ragged=True)
        # regather ragged E rows to base-partition 0 (matmul operand constraint)
        Erag2_sb = konst.tile([1, 8, 2, BPC, NP_], BF16, name="Erag2_sb")
        for hh in range(8):
            nc.sync.dma_start(out=Erag2_sb[0:1, hh], in_=Erag_sb[hh:hh + 1])

        # ---- phase 2c + 3: per head: dots, conv, attention out
        for hh in range(8):
            p0 = (hh % 2) * 64
            kft = 4 + hh // 2
            qft = hh // 2
            E_units = []
            for jt in range(2):
                D_sb = dpool.tile([128, 2, BPC, NP_], BF16, name="D_sb", tag="D")
                for b in range(BPC):
                    for s in range(2):
                        ps = ppool.tile([128, NP_], F32, name="psd", tag="ps")
                        nc.tensor.matmul(
                            ps,
                            lhsT=QK_sb[p0:p0 + 64, b, s, kft, jt * 128:(jt + 1) * 128],
                            rhs=QK_sb[p0:p0 + 64, b, s, qft, :],
                            start=True, stop=True)
                        nc.scalar.activation(D_sb[:, s, b, :], ps, AF.Copy,
                                             bias=0.0, scale=1.0)
                E_sb = dpool.tile([128, 2, BPC, NP_], BF16, name="E_sb", tag="E")
                conv_unit.cur_h = hh
                conv_unit(D_sb[:, 0, :, :], D_sb[:, 1, :, :],
                          lambda s, E_sb=E_sb: E_sb[:, s, :, :], ragged=False)
                E_units.append(E_sb)
            for b in range(BPC):
                for s in range(2):
                    po = ppool.tile([65, NP_], F32, name="pso", tag="ps")
                    nc.tensor.matmul(po, lhsT=v_sb[:, b, s, 0, hh, :],
                                     rhs=E_units[0][:, s, b, :], start=True, stop=False)
                    nc.tensor.matmul(po, lhsT=v_sb[:, b, s, 1, hh, :],
                                     rhs=E_units[1][:, s, b, :], start=False, stop=False)
                    nc.tensor.matmul(po, lhsT=v_sb[0:1, b, s, 2, hh, :],
                                     rhs=Erag2_sb[0:1, hh, s, b, :], start=False,
                                     stop=True)
                    lnr = rpool.tile([1, NP_], F32, name="lnr", tag="rcp")
                    nc.scalar.activation(lnr, po[64:65, :], AF.Ln, bias=0.0,
                                         scale=1.0)
                    rcp = rpool.tile([1, NP_], F32, name="rcp", tag="rcp")
                    nc.scalar.activation(rcp, lnr, AF.Exp, bias=0.0, scale=-1.0)
                    osb = rpool.tile([64, NP_], F32, name="osb", tag="osb")
                    nc.scalar.activation(osb, po[0:64, :], AF.Copy, bias=0.0,
                                         scale=1.0)
                    rb = ppool.tile([64, NP_], F32, name="rb", tag="ps")
                    nc.tensor.matmul(rb, lhsT=onesf_sb, rhs=rcp, start=True,
                                     stop=True)
                    nc.vector.scalar_tensor_tensor(
                        outT_sb[p0:p0 + 64, b, s, hh // 2, :],
                        in0=osb, scalar=0.0, in1=rb,
                        op0=OP.bypass, op1=OP.mult)

        # ---- phase 4: MLP + store (s-outer so wm streams once per s)
        for s in range(2):
            wm_sb = xpool.tile([128, 4, 512], BF16, name="wm_sb", tag="wv", bufs=1)
            nc.sync.dma_start(out=wm_sb, in_=wm[s].rearrange("t p f -> p t f"))
            for b in range(BPC):
                for ft in range(4):
                    ps = ppool.tile([128, NP_], F32, name="psm", tag="ps")
                    for dt in range(4):
                        nc.tensor.matmul(
                            ps, lhsT=wm_sb[:, dt, ft * 128:(ft + 1) * 128],
                            rhs=outT_sb[:, b, s, dt, :], start=(dt == 0), stop=(dt == 3))
                    rs = rpool.tile([128, NP_], F32, name="rs", tag="rs")
                    nc.scalar.activation(rs, ps, AF.Identity,
                                         bias=bmv_sb[:, s, ft:ft + 1], scale=1.0)
                    nc.sync.dma_start(out=res[s, b, ft], in_=rs)

        rpool.release(); tpool.release(); dpool.release()
        xpool.release(); ppool.release(); konst.release()
    return nc


# ----------------------------------------------------------------- frontend
_cache = {}


def kernel(**inputs):
    inputs = {k: np.asarray(v) for k, v in inputs.items()}
    cc = _fold_consts(inputs)

    # host-side packing
    SC = np.float64(SCALE)
    Wq = np.asarray(inputs['Wqkv'], np.float64)
    Wq1 = np.asarray(inputs['Wqkv1'], np.float64)
    bq = np.asarray(inputs['bqkv'], np.float64)
    bq1 = np.asarray(inputs['bqkv1'], np.float64)
    Wmlp = np.asarray(inputs['Wmlp'], np.float64)
    Wmlp1 = np.asarray(inputs['Wmlp1'], np.float64)
    bmlp = np.asarray(inputs['bmlp'], np.float64)
    bmlp1 = np.asarray(inputs['bmlp1'], np.float64)

    wqk_np = np.stack([
        np.concatenate([Wq[:, 0:512] * SC, Wq[:, 512:1024]], 1),
        np.concatenate([Wq1[:, 0:512] * SC, Wq1[:, 512:1024]], 1),
    ]).reshape(2, 4, 128, 1024).astype(ml_dtypes.bfloat16)
    wv_np = np.stack([Wq[:, 1024:1536], Wq1[:, 1024:1536]]).reshape(
        2, 4, 128, 512).astype(ml_dtypes.bfloat16)
    wm_np = np.stack([Wmlp, Wmlp1]).reshape(2, 4, 128, 512).astype(ml_dtypes.bfloat16)
    qkb_np = np.stack([
        np.concatenate([bq[0:512] * SC, bq[512:1024]]),
        np.concatenate([bq1[0:512] * SC, bq1[512:1024]]),
    ]).reshape(2, 8, 128).transpose(2, 0, 1).astype(np.float32).copy()
    bm_eff = np.stack([bq[1024:1536] @ Wmlp + bmlp,
                       bq1[1024:1536] @ Wmlp1 + bmlp1])
    bmv_np = bm_eff.reshape(2, 4, 128).transpose(2, 0, 1).astype(np.float32).copy()
    cvc_np = np.concatenate([
        cc['rho_r'], cc['m_r'], cc['chat'],
        cc['rho9_r'][:, None], cc['m9_r'][:, None], cc['const0'][:, None],
    ], axis=1).astype(np.float32)  # [8, 27]
    cvc_np = np.concatenate([cvc_np, np.zeros((8, 1), np.float32)], axis=1)

    x = np.asarray(inputs['x'], np.float32)
    l = np.asarray(inputs['l'], np.float32)
    xpad = np.zeros((2, B, D, NP_), np.float32)
    xpad[0, :, :, :N] = x.transpose(0, 2, 1)
    xpad[1, :, :, :N] = l.transpose(0, 2, 1)
    xt_all = xpad.reshape(2, B, 4, 128, NP_).astype(ml_dtypes.bfloat16)

    key = hashlib.sha256()
    for nm in ('conv1_w', 'conv1_b', 'bn_g', 'bn_b', 'conv2_w', 'conv2_b'):
        key.update(np.ascontiguousarray(inputs[nm]).tobytes())
    key = key.hexdigest()
    if key not in _cache:
        _cache[key] = _build(cc)
    nc = _cache[key]

    in_maps = []
    for c in range(NCORES):
        bs = slice(c * BPC, (c + 1) * BPC)
        in_maps.append({
            "xt": np.ascontiguousarray(xt_all[:, bs]),
            "wqk": wqk_np, "wv": wv_np, "wm": wm_np,
            "qkb": qkb_np, "bmv": bmv_np, "cvc": cvc_np,
        })
    globals()['_last_in_maps'] = in_maps
    rr = run_bass_kernel_spmd(nc, in_maps, core_ids=list(range(NCORES)))
    out0 = np.empty((B, N, D), np.float32)
    out1 = np.empty((B, N, D), np.float32)
    for c in range(NCORES):
        r = rr.results[c]["res"]            # [2, BPC, 4, 128, NP_]
        r = r.reshape(2, BPC, D, NP_)[:, :, :, :N].transpose(0, 1, 3, 2)
        out0[c * BPC:(c + 1) * BPC] = r[0]
        out1[c * BPC:(c + 1) * BPC] = r[1]
    return out0, out1


# revision 17
# speedup vs baseline: 29.0513x; 1.1636x over previous
"""Bass/Tile TRN2 kernel for nn_Encoder_55233279426649 (dual-stream encoder
block with cross-stream attention-map conv).

Sharding: data-parallel over batch — 32 batches -> 8 NeuronCores x 4 batches.
Inside each core everything runs in "feature-major" (transposed) layouts so
only host-side transposes are needed. All matmuls run in bf16 on the PE
(fp32 PSUM accumulation); the attention-map conv block is decomposed as
  sup = mu*(A*D0 + B*D1 + Cd) + nu*sum_o w2_o |z_o| + c2,   mu=(1+l)/2, nu=(1-l)/2
(LeakyReLU(z) = mu*z + nu*|z|), evaluated with fused DVE ops over
batch-packed tiles. Softmax has no max-subtract (logits are O(1)); the
denominator comes free from an appended ones-column in V.
"""
import hashlib
import numpy as np
import ml_dtypes

import concourse.bass as bass
import concourse.mybir as mybir
import concourse.tile as tile
from concourse.vector_clock import ScopedClock
from concourse.bass_utils import run_bass_kernel_spmd

# ---------------------------------------------------------------- constants
B, N, D, H, NA = 32, 257, 512, 8, 8
DH = D // H
SCALE = (D / H) ** -0.5
BN_EPS = 1e-5
LAM = 0.01
MU = (1 + LAM) / 2
NU = (1 - LAM) / 2
NP_ = 258            # padded query length (even for DVE packed modes)
NCORES = 8
BPC = B // NCORES    # batches per core
F32 = mybir.dt.float32
BF16 = mybir.dt.bfloat16
AF = mybir.ActivationFunctionType
OP = mybir.AluOpType

# ------------------------------------------------- walrus 1-wait legalizer
_ctr = [0]


def _mk_wait_nop(engine, wait):
    _ctr[0] += 1
    nop = mybir.InstNoOp(name=f"Iws-{_ctr[0]}", engine=engine, ins=[], outs=[])
    nop.sync_info = mybir.SyncInfo(on_wait=[wait], on_update=[])
    return nop


class FixedTileContext(tile.TileContext):
    """Splits >1-wait instructions into wait-carrying nops (this container's
    walrus accepts at most one sync-wait command per instruction)."""

    def _lower_ordered_insts(self, postordered_blocks):
        for bb_name in list(postordered_blocks.keys()):
            insts = postordered_blocks[bb_name]
            new = []
            changed = False
            for inst in insts:
                si = inst.sync_info
                if si is not None and si.on_wait is not None and len(si.on_wait) > 1:
                    waits = list(si.on_wait)
                    for w in waits[:-1]:
                        new.append(_mk_wait_nop(inst.engine, w))
                    si.on_wait = [waits[-1]]
                    changed = True
                new.append(inst)
            if changed:
                if isinstance(insts, list):
                    insts[:] = new
                else:
                    postordered_blocks[bb_name] = new
        return super()._lower_ordered_insts(postordered_blocks)

    def _drain_and_barrier(self, tick_clock, wait_clock):
        nc = self.nc
        drain_inst = nc.sync.drain()
        wait_clock.add_sem_waits(
            drain_inst.ins, ScopedClock({None: tick_clock.global_clock})
        )
        si = drain_inst.ins.sync_info
        if si is not None and si.on_wait is not None and len(si.on_wait) > 1:
            waits = list(si.on_wait)
            si.on_wait = waits[:1]
            for w in waits[1:]:
                d2 = nc.sync.drain()
                si2 = d2.ins.sync_info
                if si2 is None:
                    d2.ins.sync_info = mybir.SyncInfo(on_wait=[w], on_update=[])
                else:
                    si2.on_wait = list(si2.on_wait or []) + [w]
        nc.all_engine_barrier()
        assert self.sems is not None
        popped = nc._tile_sem_poison_stack.pop()
        assert popped is self._sem_poison
        nc.clear_and_free_semaphores(list(self.sems.allocated().values()))
        nc.all_engine_barrier()


# ------------------------------------------------------------- host folding
def _fold_consts(inputs):
    """Returns dict of host-folded constants (f64 where it matters)."""
    conv1_w = np.asarray(inputs['conv1_w'], np.float64)
    conv1_b = np.asarray(inputs['conv1_b'], np.float64)
    bn_g = np.asarray(inputs['bn_g'], np.float64)
    bn_b = np.asarray(inputs['bn_b'], np.float64)
    conv2_w = np.asarray(inputs['conv2_w'], np.float64)
    conv2_b = np.asarray(inputs['conv2_b'], np.float64)
    inv = 1.0 / np.sqrt(1.0 + BN_EPS)
    g = conv1_w[:, :, 0] * bn_g * inv          # [H, NA] coef on D0 (dots)
    h = conv1_w[:, :, 1] * bn_g * inv          # coef on D1 (dots1)
    d = conv1_b * bn_g * inv + bn_b            # [H, NA]
    w2 = conv2_w
    A = (w2 * g).sum(1)
    Bc = (w2 * h).sum(1)
    Cd = (w2 * d).sum(1)
    const0 = MU * Cd + conv2_b                 # exp bias per head
    eps = 1e-30
    piv_is_h = np.abs(h) >= np.abs(g)
    hs = np.where(np.abs(h) < eps, eps, h)
    gs = np.where(np.abs(g) < eps, eps, g)
    rho = np.where(piv_is_h, g / hs, h / gs)
    m = np.where(piv_is_h, hs, gs)
    chat = NU * w2
    As = np.where(np.abs(A) < eps, eps, A)
    Bs = np.where(np.abs(Bc) < eps, eps, Bc)
    piv9_is_B = np.abs(Bc) >= np.abs(A)
    rho9 = np.where(piv9_is_B, A / Bs, Bc / As)
    m9 = np.where(piv9_is_B, Bs, As) * MU
    # ragged (fixed pivot = h / B)
    rho_r = g / hs
    m_r = hs
    rho9_r = A / Bs
    m9_r = Bs * MU
    # full units use the relu form: LReLU(z) = lam*z + (1-lam)*relu(z)
    c_full = (1 - LAM) * w2                       # [H, NA]
    M_full = c_full * m                           # scalar1 for opB
    s2_full = -c_full * d                         # scalar2 for opB (max/min shift)
    is_max = w2 >= 0
    m9_lam = np.where(piv9_is_B, Bs, As) * LAM
    const0_full = LAM * Cd + conv2_b + (c_full * d).sum(1)
    return dict(g=g, h=h, d=d, piv_is_h=piv_is_h, rho=rho, m=m, chat=chat,
                rho9=rho9, m9=m9, piv9_is_B=piv9_is_B, const0=const0,
                rho_r=rho_r, m_r=m_r, rho9_r=rho9_r, m9_r=m9_r,
                M_full=M_full, s2_full=s2_full, is_max=is_max,
                m9_lam=m9_lam, const0_full=const0_full)


# ------------------------------------------------------------- bass builder
def _build(cc):
    """cc: dict of folded conv consts (floats embedded as immediates)."""
    nc = bass.Bass()
    xt = nc.dram_tensor("xt", [2, BPC, 4, 128, NP_], BF16, kind="ExternalInput")
    wqk = nc.dram_tensor("wqk", [2, 4, 128, 1024], BF16, kind="ExternalInput")
    wv = nc.dram_tensor("wv", [2, 4, 128, 512], BF16, kind="ExternalInput")
    wm = nc.dram_tensor("wm", [2, 4, 128, 512], BF16, kind="ExternalInput")
    qkb = nc.dram_tensor("qkb", [128, 2, 8], F32, kind="ExternalInput")
    bmv = nc.dram_tensor("bmv", [128, 2, 4], F32, kind="ExternalInput")
    cvc = nc.dram_tensor("cvc", [8, 28], F32, kind="ExternalInput")
    res = nc.dram_tensor("res", [2, BPC, 4, 128, NP_], F32, kind="ExternalOutput")

    with FixedTileContext(nc) as tc:
        konst = tc.alloc_tile_pool(name="konst", bufs=1)
        ppool = tc.alloc_tile_pool(name="ppool", bufs=8, space="PSUM")
        xpool = tc.alloc_tile_pool(name="xpool", bufs=2)
        dpool = tc.alloc_tile_pool(name="dpool", bufs=2)
        tpool = tc.alloc_tile_pool(name="tpool", bufs=2)
        rpool = tc.alloc_tile_pool(name="rpool", bufs=2)

        # ---- resident constants/weights
        qkb_sb = konst.tile([128, 2, 8], F32, name="qkb_sb")
        nc.sync.dma_start(out=qkb_sb, in_=qkb[:, :, :])
        bmv_sb = konst.tile([128, 2, 4], F32, name="bmv_sb")
        nc.sync.dma_start(out=bmv_sb, in_=bmv[:, :, :])
        cvc_sb = konst.tile([8, 28], F32, name="cvc_sb")
        nc.sync.dma_start(out=cvc_sb, in_=cvc[:, :])
        ones_sb = konst.tile([1, 64], BF16, name="ones_sb")
        nc.vector.memset(ones_sb, 1.0)
        onesf_sb = konst.tile([1, 64], F32, name="onesf_sb")
        nc.vector.memset(onesf_sb, 1.0)

        QK_sb = konst.tile([128, BPC, 2, 8, NP_], BF16, name="QK_sb")
        v_sb = konst.tile([128, BPC, 2, 3, 8, 65], BF16, name="v_sb")
        outT_sb = konst.tile([128, BPC, 2, 4, NP_], BF16, name="outT_sb")
        rag_sb = konst.tile([40, 2, BPC, NP_], BF16, name="rag_sb")
        Drag_sb = rag_sb[0:8]
        Erag_sb = rag_sb[32:40]

        # ---- phase 1: projections (s-outer so weights stream once per s)
        for s in range(2):
            wqk_sb = xpool.tile([128, 4, 1024], BF16, name="wqk_sb", tag="wqk", bufs=1)
            nc.sync.dma_start(out=wqk_sb, in_=wqk[s].rearrange("t p f -> p t f"))
            wv_sb = xpool.tile([128, 4, 512], BF16, name="wv_sb", tag="wv", bufs=1)
            nc.sync.dma_start(out=wv_sb, in_=wv[s].rearrange("t p f -> p t f"))
            for b in range(BPC):
                xts = xpool.tile([128, 4, NP_], BF16, name="xts", tag="xts")
                nc.sync.dma_start(out=xts, in_=xt[s, b].rearrange("t p i -> p t i"))
                for ft in range(8):
                    ps = ppool.tile([128, NP_], F32, name="psqk", tag="ps")
                    for dt in range(4):
                        nc.tensor.matmul(
                            ps, lhsT=wqk_sb[:, dt, ft * 128:(ft + 1) * 128],
                            rhs=xts[:, dt, :], start=(dt == 0), stop=(dt == 3))
                    nc.scalar.activation(QK_sb[:, b, s, ft, :], ps, AF.Identity,
                                         bias=qkb_sb[:, s, ft:ft + 1], scale=1.0)
                for tt in range(3):
                    tsz = 128 if tt < 2 else 1
                    ps = ppool.tile([128, 512], F32, name="psv", tag="ps")
                    for dt in range(4):
                        nc.tensor.matmul(
                            ps[0:tsz, :], lhsT=xts[:, dt, tt * 128:tt * 128 + tsz],
                            rhs=wv_sb[:, dt, :], start=(dt == 0), stop=(dt == 3))
                    nc.scalar.activation(
                        v_sb[0:tsz, b, s, tt, :, 0:64],
                        ps[0:tsz, :].rearrange("p (h e) -> p h e", h=8),
                        AF.Copy, bias=0.0, scale=1.0)
                    nc.vector.memset(v_sb[0:tsz, b, s, tt, :, 64:65], 1.0)

        # ---- phase 2a: ragged (j=256) dots for all heads -> Drag
        Dragf_sb = konst.tile([1, 8, 2, BPC, NP_], BF16, name="Dragf_sb")
        for b in range(BPC):
            for s in range(2):
                for hh in range(8):
                    p0 = (hh % 2) * 64
                    kft = 4 + hh // 2
                    qft = hh // 2
                    psr = ppool.tile([1, NP_], F32, name="psr", tag="ps")
                    nc.tensor.matmul(
                        psr,
                        lhsT=QK_sb[p0:p0 + 64, b, s, kft, 256:257],
                        rhs=QK_sb[p0:p0 + 64, b, s, qft, :],
                        start=True, stop=True)
                    nc.scalar.activation(Dragf_sb[0:1, hh, s, b, :], psr, AF.Copy,
                                         bias=0.0, scale=1.0)
        for hh in range(8):
            nc.sync.dma_start(out=Drag_sb[hh:hh + 1], in_=Dragf_sb[0:1, hh])

        # ---- phase 2b: ragged conv unit (per-partition consts over heads)
        def conv_unit(D0ap, D1ap, outEap, ragged):
            """D0ap/D1ap: [P, BPC, NP_] bf16 views. outEap(s) -> E view."""
            if ragged:
                def sc(col):
                    return cvc_sb[:, col:col + 1]
                rho_o = [sc(o) for o in range(8)]
                m_o = [sc(8 + o) for o in range(8)]
                chat_o = [sc(16 + o) for o in range(8)]
                rho9v, m9v, expb = sc(24), sc(25), sc(26)
                d_o = [sc(27) for o in range(8)]
                piv = [True] * 8
                piv9 = True
            else:
                hh = conv_unit.cur_h
                rho_o = [float(cc['rho'][hh, o]) for o in range(8)]
                M_o = [float(cc['M_full'][hh, o]) for o in range(8)]
                s2_o = [float(cc['s2_full'][hh, o]) for o in range(8)]
                ismax_o = [bool(cc['is_max'][hh, o]) for o in range(8)]
                rho9v = float(cc['rho9'][hh]); m9v = float(cc['m9_lam'][hh])
                expb = float(cc['const0_full'][hh])
                piv = [bool(cc['piv_is_h'][hh, o]) for o in range(8)]
                piv9 = bool(cc['piv9_is_B'][hh])
            P = D0ap.shape[0]
            sh = list(D0ap.shape)
            i0, i1 = (D0ap, D1ap) if piv9 else (D1ap, D0ap)
            t9 = tpool.tile(sh, BF16, name="t9", tag="tau", bufs=1)
            nc.vector.scalar_tensor_tensor(t9, in0=i0, scalar=rho9v, in1=i1,
                                           op0=OP.mult, op1=OP.add)
            acc = tpool.tile(sh, BF16, name="acc", tag="acc")
            nc.vector.tensor_scalar(acc, in0=t9, scalar1=m9v, scalar2=None,
                                    op0=OP.mult)
            for o in range(8):
                i0o, i1o = (D0ap, D1ap) if piv[o] else (D1ap, D0ap)
                tau = tpool.tile(sh, BF16, name="tau", tag="tau", bufs=1)
                nc.vector.scalar_tensor_tensor(tau, in0=i0o, scalar=rho_o[o],
                                               in1=i1o, op0=OP.mult, op1=OP.add)
                if ragged:
                    u1 = tpool.tile(sh, BF16, name="u1", tag="u", bufs=2)
                    nc.vector.tensor_scalar(u1, in0=tau, scalar1=m_o[o],
                                            scalar2=d_o[o], op0=OP.mult,
                                            op1=OP.add)
                    u = tpool.tile(sh, BF16, name="u", tag="u", bufs=2)
                    nc.vector.scalar_tensor_tensor(u, in0=u1, scalar=-1.0,
                                                   in1=u1, op0=OP.mult,
                                                   op1=OP.max)
                    acc2 = tpool.tile(sh, BF16, name="acc2", tag="acc")
                    nc.vector.scalar_tensor_tensor(acc2, in0=u, scalar=chat_o[o],
                                                   in1=acc, op0=OP.mult,
                                                   op1=OP.add)
                else:
                    u = tpool.tile(sh, BF16, name="u", tag="u", bufs=2)
                    nc.vector.tensor_scalar(u, in0=tau, scalar1=M_o[o],
                                            scalar2=s2_o[o], op0=OP.mult,
                                            op1=OP.max if ismax_o[o] else OP.min)
                    acc2 = tpool.tile(sh, BF16, name="acc2", tag="acc")
                    nc.vector.tensor_add(acc2, u, acc)
                acc = acc2
            for s in range(2):
                Ds = D0ap if s == 0 else D1ap
                lg = tpool.tile(sh, BF16, name="lg", tag="u", bufs=2)
                nc.vector.tensor_add(lg, Ds, acc)
                nc.scalar.activation(outEap(s), lg, AF.Exp, bias=expb, scale=1.0)

        conv_unit.cur_h = None
        conv_unit(Drag_sb[:, 0, :, :], Drag_sb[:, 1, :, :],
                  lambda s: Erag_sb[:, s, :, :], ragged=True)
        # regather ragged E rows to base-partition 0 (matmul operand constraint)
        Erag2_sb = konst.tile([1, 8, 2, BPC, NP_], BF16, name="Erag2_sb")
        for hh in range(8):
            nc.sync.dma_start(out=Erag2_sb[0:1, hh], in_=Erag_sb[hh:hh + 1])

        # ---- phase 2c + 3: per head: dots, conv, attention out
        for hh in range(8):
            p0 = (hh % 2) * 64
            kft = 4 + hh // 2
            qft = hh // 2
            D_sb = dpool.tile([128, 2, 2, BPC, NP_], BF16, name="D_sb", tag="D",
                              bufs=1)
            for jt in range(2):
                for b in range(BPC):
                    for s in range(2):
                        ps = ppool.tile([128, NP_], F32, name="psd", tag="ps")
                        nc.tensor.matmul(
                            ps,
                            lhsT=QK_sb[p0:p0 + 64, b, s, kft, jt * 128:(jt + 1) * 128],
                            rhs=QK_sb[p0:p0 + 64, b, s, qft, :],
                            start=True, stop=True)
                        nc.scalar.activation(D_sb[:, jt, s, b, :], ps, AF.Copy,
                                             bias=0.0, scale=1.0)
            E_sb = dpool.tile([128, 2, 2, BPC, NP_], BF16, name="E_sb", tag="E",
                              bufs=1)
            conv_unit.cur_h = hh
            conv_unit(D_sb[:, :, 0, :, :], D_sb[:, :, 1, :, :],
                      lambda s, E_sb=E_sb: E_sb[:, :, s, :, :], ragged=False)
            for b in range(BPC):
                for s in range(2):
                    po = ppool.tile([65, NP_], F32, name="pso", tag="ps")
                    nc.tensor.matmul(po, lhsT=v_sb[:, b, s, 0, hh, :],
                                     rhs=E_sb[:, 0, s, b, :], start=True, stop=False)
                    nc.tensor.matmul(po, lhsT=v_sb[:, b, s, 1, hh, :],
                                     rhs=E_sb[:, 1, s, b, :], start=False, stop=False)
                    nc.tensor.matmul(po, lhsT=v_sb[0:1, b, s, 2, hh, :],
                                     rhs=Erag2_sb[0:1, hh, s, b, :], start=False,
                                     stop=True)
                    lnr = rpool.tile([1, NP_], F32, name="lnr", tag="rcp")
                    nc.scalar.activation(lnr, po[64:65, :], AF.Ln, bias=0.0,
                                         scale=1.0)
                    rcp = rpool.tile([1, NP_], F32, name="rcp", tag="rcp")
                    nc.scalar.activation(rcp, lnr, AF.Exp, bias=0.0, scale=-1.0)
                    osb = rpool.tile([64, NP_], F32, name="osb", tag="osb", bufs=1)
                    nc.scalar.activation(osb, po[0:64, :], AF.Copy, bias=0.0,
                                         scale=1.0)
                    rb = ppool.tile([64, NP_], F32, name="rb", tag="ps")
                    nc.tensor.matmul(rb, lhsT=onesf_sb, rhs=rcp, start=True,
                                     stop=True)
                    nc.vector.scalar_tensor_tensor(
                        outT_sb[p0:p0 + 64, b, s, hh // 2, :],
                        in0=osb, scalar=0.0, in1=rb,
                        op0=OP.bypass, op1=OP.mult)

        # ---- phase 4: MLP + store (s-outer so wm streams once per s)
        for s in range(2):
            wm_sb = xpool.tile([128, 4, 512], BF16, name="wm_sb", tag="wv", bufs=1)
            nc.sync.dma_start(out=wm_sb, in_=wm[s].rearrange("t p f -> p t f"))
            for b in range(BPC):
                for ft in range(4):
                    ps = ppool.tile([128, NP_], F32, name="psm", tag="ps")
                    for dt in range(4):
                        nc.tensor.matmul(
                            ps, lhsT=wm_sb[:, dt, ft * 128:(ft + 1) * 128],
                            rhs=outT_sb[:, b, s, dt, :], start=(dt == 0), stop=(dt == 3))
                    rs = rpool.tile([128, NP_], F32, name="rs", tag="rs", bufs=1)
                    nc.scalar.activation(rs, ps, AF.Identity,
                                         bias=bmv_sb[:, s, ft:ft + 1], scale=1.0)
                    nc.sync.dma_start(out=res[s, b, ft], in_=rs)

        rpool.release(); tpool.release(); dpool.release()
        xpool.release(); ppool.release(); konst.release()
    return nc


# ----------------------------------------------------------------- frontend
_cache = {}


def kernel(**inputs):
    inputs = {k: np.asarray(v) for k, v in inputs.items()}
    cc = _fold_consts(inputs)

    # host-side packing
    SC = np.float64(SCALE)
    Wq = np.asarray(inputs['Wqkv'], np.float64)
    Wq1 = np.asarray(inputs['Wqkv1'], np.float64)
    bq = np.asarray(inputs['bqkv'], np.float64)
    bq1 = np.asarray(inputs['bqkv1'], np.float64)
    Wmlp = np.asarray(inputs['Wmlp'], np.float64)
    Wmlp1 = np.asarray(inputs['Wmlp1'], np.float64)
    bmlp = np.asarray(inputs['bmlp'], np.float64)
    bmlp1 = np.asarray(inputs['bmlp1'], np.float64)

    wqk_np = np.stack([
        np.concatenate([Wq[:, 0:512] * SC, Wq[:, 512:1024]], 1),
        np.concatenate([Wq1[:, 0:512] * SC, Wq1[:, 512:1024]], 1),
    ]).reshape(2, 4, 128, 1024).astype(ml_dtypes.bfloat16)
    wv_np = np.stack([Wq[:, 1024:1536], Wq1[:, 1024:1536]]).reshape(
        2, 4, 128, 512).astype(ml_dtypes.bfloat16)
    wm_np = np.stack([Wmlp, Wmlp1]).reshape(2, 4, 128, 512).astype(ml_dtypes.bfloat16)
    qkb_np = np.stack([
        np.concatenate([bq[0:512] * SC, bq[512:1024]]),
        np.concatenate([bq1[0:512] * SC, bq1[512:1024]]),
    ]).reshape(2, 8, 128).transpose(2, 0, 1).astype(np.float32).copy()
    bm_eff = np.stack([bq[1024:1536] @ Wmlp + bmlp,
                       bq1[1024:1536] @ Wmlp1 + bmlp1])
    bmv_np = bm_eff.reshape(2, 4, 128).transpose(2, 0, 1).astype(np.float32).copy()
    cvc_np = np.concatenate([
        cc['rho_r'], cc['m_r'], cc['chat'],
        cc['rho9_r'][:, None], cc['m9_r'][:, None], cc['const0'][:, None],
    ], axis=1).astype(np.float32)  # [8, 27]
    cvc_np = np.concatenate([cvc_np, np.zeros((8, 1), np.float32)], axis=1)

    x = np.asarray(inputs['x'], np.float32)
    l = np.asarray(inputs['l'], np.float32)
    xpad = np.zeros((2, B, D, NP_), np.float32)
    xpad[0, :, :, :N] = x.transpose(0, 2, 1)
    xpad[1, :, :, :N] = l.transpose(0, 2, 1)
    xt_all = xpad.reshape(2, B, 4, 128, NP_).astype(ml_dtypes.bfloat16)

    key = hashlib.sha256()
    for nm in ('conv1_w', 'conv1_b', 'bn_g', 'bn_b', 'conv2_w', 'conv2_b'):
        key.update(np.ascontiguousarray(inputs[nm]).tobytes())
    key = key.hexdigest()
    if key not in _cache:
        _cache[key] = _build(cc)
    nc = _cache[key]

    in_maps = []
    for c in range(NCORES):
        bs = slice(c * BPC, (c + 1) * BPC)
        in_maps.append({
            "xt": np.ascontiguousarray(xt_all[:, bs]),
            "wqk": wqk_np, "wv": wv_np, "wm": wm_np,
            "qkb": qkb_np, "bmv": bmv_np, "cvc": cvc_np,
        })
    globals()['_last_in_maps'] = in_maps
    rr = run_bass_kernel_spmd(nc, in_maps, core_ids=list(range(NCORES)))
    out0 = np.empty((B, N, D), np.float32)
    out1 = np.empty((B, N, D), np.float32)
    for c in range(NCORES):
        r = rr.results[c]["res"]            # [2, BPC, 4, 128, NP_]
        r = r.reshape(2, BPC, D, NP_)[:, :, :, :N].transpose(0, 1, 3, 2)
        out0[c * BPC:(c + 1) * BPC] = r[0]
        out1[c * BPC:(c + 1) * BPC] = r[1]
    return out0, out1
